# revision 33
# baseline (speedup 1.0000x reference)
"""KagomeConv2D Trainium2 Bass kernel.

Strategy (pure data parallel over batch, 8 cores x 256 batches):
  - reference = 3 masked 4x4/stride-2 convs on a zero-padded 10x10 grid with 18
    boundary-fixup copies, interleaved into an 8x8 output with 12 scatter-zeros.
  - Each masked conv has only 5 live taps; a (conv,tap) pair contributes
    W[:, :, dr, dc] @ xp[:, :, 2oh+dr, 2ow+dc] -- a [O,I] x [I, B*16] matmul.
  - Channels are split even/odd across partitions (partition p holds channels
    2p and 2p+1, likewise output channels) so every DMA descriptor moves 512
    contiguous bytes on both the HBM and SBUF side: descriptors under 512B pay
    a 2x SDMA read-modify-write penalty.
  - The PE pays ~9 cycles per rhs access-pattern run, so the rhs wants long
    contiguous runs: a naive NCHW view of the stride-2 conv gives 128 runs of
    4 sparse elements per matmul (measured 685ns/MM vs 213ns ideal).  We store
    the padded image PHASE-MAJOR with batch innermost:
        xp_ph[p, j, r%2, c%2, r//2, c//2, b]
    so each tap window is a [4, 4of5, b] slice = 4 contiguous 256B runs per
    matmul.  No separate im2col stage is needed; the batch-transpose cost sits
    in the interior copies (scattered gather reads), which are split across
    Vector and Scalar while GpSimd only does memsets.
  - Per 32-batch block: 60 matmuls (5 taps x 2 parities x 3 convs x 2 output
    parities) accumulate in 6 PSUM banks; ScalarE drains with fused bias into
    interleaved output position; scatter-zeros re-applied; one 2MB DMA out.

Weights/bias are tiny and reformatted host-side into matmul layout.
"""

import sys

sys.path.insert(0, "/opt/trn_rl_repo")

import numpy as np

import concourse.bass as bass  # noqa: E402
import concourse.bacc as bacc  # noqa: E402
import concourse.mybir as mybir  # noqa: E402
from concourse.tile import TileContext  # noqa: E402
from concourse.bass_utils import run_bass_kernel_spmd  # noqa: E402

F32 = mybir.dt.float32
BF16 = mybir.dt.bfloat16

B_FULL = 2048
N_CORES = 8
B_CORE = B_FULL // N_CORES
CIN = 256
COUT = 256

# (conv, dr, dc) pairs with mask==1. conv: 0=up, 1=left, 2=right.
# Interior taps (windows that touch no border cell) are ordered first so each
# group's first matmuls depend only on the interior copies, not the fixups.
TAPS_UP = [(1, 1), (2, 1), (2, 2), (0, 0), (0, 1)]
TAPS_LEFT = [(1, 1), (2, 1), (2, 2), (2, 0), (3, 1)]
TAPS_RIGHT = [(1, 1), (2, 1), (2, 2), (2, 3), (3, 3)]
# Note: no tap has (dr%2, dc%2) == (1, 0) — that quarter of the padded image
# is never read, so its interior copies and zero-inits are skipped.
USED_PHASES = [(1, 1), (0, 1), (0, 0)]
CONV_TAPS = [TAPS_UP, TAPS_LEFT, TAPS_RIGHT]
PAIRS = [(ci, dr, dc) for ci, taps in enumerate(CONV_TAPS) for (dr, dc) in taps]
NPAIRS = len(PAIRS)  # 15

# Boundary fixups on the zero-padded 10x10 grid (xp coords); sources are all
# interior cells, given here in 8x8 x coords (xp minus 1).
DST_R = [0, 0, 0, 0, 1, 2, 3, 4, 6, 7, 8, 9, 9, 9, 8, 6, 4, 2]
DST_C = [0, 1, 2, 3, 5, 6, 7, 8, 9, 9, 9, 9, 7, 5, 3, 1, 0, 0]
SRC_R = [7, 7, 7, 7, 4, 5, 6, 7, 1, 2, 3, 4, 0, 0, 3, 1, 7, 5]
SRC_C = [3, 4, 5, 6, 0, 1, 2, 3, 0, 0, 0, 0, 2, 0, 6, 4, 7, 7]


def _phase_fixup_runs():
    """Group the 18 boundary copies into strided runs in phase-major coords.

    Returns [(hl, wl, d0, dstep, s0, sstep, cnt)] where d indexes the flat
    5x5 phase grid and s the flat 8x8 x image.
    """
    by_phase = {}
    for r, c, sr, sc in zip(DST_R, DST_C, SRC_R, SRC_C):
        key = (r % 2, c % 2)
        by_phase.setdefault(key, []).append((5 * (r // 2) + c // 2, 8 * sr + sc))
    runs = []
    for (hl, wl), cells in by_phase.items():
        i = 0
        while i < len(cells):
            d0, s0 = cells[i]
            cnt = 1
            if i + 1 < len(cells):
                dd = cells[i + 1][0] - d0
                ds = cells[i + 1][1] - s0
                if dd > 0:
                    while (i + cnt < len(cells)
                           and cells[i + cnt][0] == d0 + cnt * dd
                           and cells[i + cnt][1] == s0 + cnt * ds):
                        cnt += 1
            if cnt == 1:
                dd, ds = 1, 1
            runs.append((hl, wl, d0, dd, s0, ds, cnt))
            i += cnt
    return runs


PHASE_FIXUPS = _phase_fixup_runs()

# Border cells of xp that stay zero, as (r, c) on the 10x10 grid.
XP_ZERO_CELLS = (
    [(0, c) for c in range(4, 10)] + [(9, c) for c in range(0, 5)]
    + [(9, 6), (9, 8)] + [(1, 0), (3, 0)]
    + [(r, 0) for r in range(5, 9)] + [(r, 9) for r in range(1, 6)]
)


def _phase_zero_runs():
    """Group the zero border cells into strided runs on the flat 5x5 grids."""
    by_phase = {}
    for r, c in XP_ZERO_CELLS:
        by_phase.setdefault((r % 2, c % 2), []).append(5 * (r // 2) + c // 2)
    runs = []
    for (hl, wl), cells in by_phase.items():
        cells = sorted(set(cells))
        i = 0
        while i < len(cells):
            d0 = cells[i]
            cnt = 1
            dd = 1
            if i + 1 < len(cells):
                dd = cells[i + 1] - d0
                while (i + cnt < len(cells)
                       and cells[i + cnt] == d0 + cnt * dd):
                    cnt += 1
            if cnt == 1:
                dd = 1
            runs.append((hl, wl, d0, dd, cnt))
            i += cnt
    return runs


PHASE_ZEROS = _phase_zero_runs()

# Final scatter-zero positions on the 8x8 output (flat offsets), grouped.
OUT_ZERO_RUNS = [
    (4, 8, 2),     # (0,4),(0,6)
    (13, 16, 1),   # (1,5),(1,6),(1,7)
    (22, 23, 1),   # (2,6)
    (31, 32, 1),   # (3,7)
    (40, 57, 8),   # (5,0),(6,0),(7,0)
    (57, 59, 1),   # (7,1),(7,2)
]


def build_nc(b_core=B_CORE, bsub=32, xsb_bufs=4, psum_bufs=8):
    """Build the single-core Bass program (same NEFF runs SPMD on all cores)."""
    assert b_core % bsub == 0
    nblocks = b_core // bsub
    n_free = bsub * 16
    assert n_free <= 512

    nc = bacc.Bacc(
        "TRN2",
        target_bir_lowering=False,
        debug=False,
        enable_asserts=False,
    )
    x = nc.dram_tensor("x", (b_core, CIN, 8, 8), F32, kind="ExternalInput")
    wt = nc.dram_tensor("wt", (128, NPAIRS, 2, 2, 128), BF16, kind="ExternalInput")
    bias = nc.dram_tensor("bias", (128, 128), F32, kind="ExternalInput")
    out = nc.dram_tensor("out", (b_core, COUT, 8, 8), F32, kind="ExternalOutput")
    xap, wap, bap, oap = x.ap(), wt.ap(), bias.ap(), out.ap()

    with TileContext(nc) as tc:
        with (
            tc.tile_pool(name="const", bufs=1) as cpool,
            tc.tile_pool(name="xin", bufs=xsb_bufs) as xpool,
            tc.tile_pool(name="xp", bufs=1) as xppool,
            tc.tile_pool(name="osb", bufs=1) as opool,
            tc.tile_pool(name="ps", bufs=psum_bufs, space="PSUM") as pspool,
        ):
            # Block-0 input DMA first: it gates the critical path (assembly
            # then first matmuls); weights stream concurrently on the scalar
            # HWDGE ring, split per conv so the first matmul group only waits
            # for the 5 "up" pairs.
            # Ring plan: the sync HWDGE ring owns the input stream in FIFO
            # order [wt-up, x0-half0, x1..x7] (a ring's FIFO is natural
            # priority; descriptor generation is ~2.5ns/descriptor), the
            # scalar ring owns [x0-half1, bias, wt-left/right, outputs...].
            # x0 is split across both rings to halve the startup latency.
            w_sb = cpool.tile([128, NPAIRS, 2, 2, 128], BF16, name="w_sb")
            nc.sync.dma_start(out=w_sb[:, 0:5], in_=wap[:, 0:5])
            x_sb0 = xpool.tile([128, bsub, 2, 64], F32, name="x_sb", tag="x_sb")
            bh0 = bsub // 2
            for h, ring in ((0, nc.sync), (1, nc.scalar)):
                ring.dma_start(
                    out=x_sb0[:, h * bh0 : (h + 1) * bh0],
                    in_=xap[h * bh0 : (h + 1) * bh0].rearrange(
                        "b (p j) h w -> p b j (h w)", j=2
                    ),
                )
            bias_sb = cpool.tile([128, 128], F32, name="bias_sb")
            nc.scalar.dma_start(out=bias_sb[:], in_=bap)
            nc.scalar.dma_start(out=w_sb[:, 5:10], in_=wap[:, 5:10])
            # Right-conv weights aren't needed until the 5th matmul group;
            # gate them on x0's second half so they don't compete with it.
            nc.vector.tensor_copy(
                out=w_sb[0:1, 10:11, 0:1, 0:1, 0:1],
                in_=x_sb0[0:1, 0:1, 0:1, 0:1],
            )
            nc.scalar.dma_start(out=w_sb[:, 10:15], in_=wap[:, 10:15])

            # Persistent double-buffered phase-major xp tiles:
            # [p, parity j, r%2, c%2, r//2, c//2, b]; xp_ph[p, j] is the padded
            # 10x10 image of input channel 2p+j, batch innermost. The final
            # 32-batch block is split into two 16-batch blocks (own narrow
            # tiles, so the (k,b) rhs runs stay contiguous) — this halves the
            # final output transfer hanging off the end of the kernel.
            xp_tiles = [
                xppool.tile([128, 2, 2, 2, 5, 5, bsub], BF16,
                            name=f"xpt{i}", tag=f"xpt{i}")
                for i in range(2)
            ]
            xp16_tiles = [
                xppool.tile([128, 2, 2, 2, 5, 5, bsub // 2], BF16,
                            name=f"xpt16_{i}", tag=f"xpt16_{i}")
                for i in range(2)
            ]
            # Out staging tiles: [128p, b, parity g, 64]; osb[p, :, g] is
            # output channel 2p+g. (b, g, hw) contiguous -> 512B DMA runs.
            osb_tiles = [
                opool.tile([128, bsub, 2, 64], F32, name=f"osb{i}", tag=f"osb{i}")
                for i in range(2)
            ]
            x_sb16_tiles = [
                opool.tile([128, bsub // 2, 2, 64], F32, name=f"xs16_{i}",
                           tag=f"xs16_{i}")
                for i in range(2)
            ]

            # One-time zero init of the xp border cells that stay zero, as
            # strided runs on the flat 5x5 phase grids (tiny, off the
            # critical path engines where possible).
            for ti, xp in enumerate(xp_tiles + xp16_tiles):
                xpz = xp.rearrange("p j hl wl i k b -> p j hl wl (i k) b")
                eng = nc.vector if ti == 0 else nc.gpsimd
                for (hl, wl, d0, dd, cnt) in PHASE_ZEROS:
                    if (hl, wl) not in USED_PHASES:
                        continue
                    eng.memset(
                        xpz[:, :, hl, wl, d0 : d0 + dd * (cnt - 1) + 1 : dd, :], 0.0
                    )
            # One-time zero init: structural zeros (even row, odd col).
            for osb in osb_tiles:
                ob = osb.rearrange("p b g (h w) -> p b g h w", w=8)
                nc.gpsimd.memset(ob[:, :, :, 0:8:2, 1:8:2], 0.0)

            # Block plan: 7 full 32-batch blocks, then two 16-batch blocks.
            plan = [(q * bsub, bsub) for q in range(nblocks - 1)]
            plan += [((nblocks - 1) * bsub, bsub // 2),
                     ((nblocks - 1) * bsub + bsub // 2, bsub // 2)]
            xsb_hist = [x_sb0]

            for blk, (b0, bs) in enumerate(plan):
                # [128p, b, parity j, 64]; (j, hw) contiguous = 512B runs on
                # both the HBM and SBUF side of the DMA.
                if blk == 0:
                    x_sb = x_sb0
                else:
                    if bs == bsub:
                        x_sb = xpool.tile([128, bsub, 2, 64], F32,
                                          name="x_sb", tag="x_sb")
                    else:
                        x_sb = x_sb16_tiles[blk - (nblocks - 1)]
                    # Throttle the prefetch: a 1-element touch makes this DMA
                    # wait for the DMA two blocks back, so at most ~2 input
                    # transfers are in flight and the SDMA engines' fair
                    # round-robin can't starve the oldest (critical) one.
                    gate = xsb_hist[max(0, blk - 2)]
                    nc.vector.tensor_copy(
                        out=x_sb[0:1, 0:1, 0:1, 0:1], in_=gate[0:1, 0:1, 0:1, 0:1]
                    )
                    nc.sync.dma_start(
                        out=x_sb[:],
                        in_=xap[b0 : b0 + bs].rearrange(
                            "b (p j) h w -> p b j (h w)", j=2
                        ),
                    )
                    xsb_hist.append(x_sb)

                xp = (xp_tiles[blk % 2] if bs == bsub
                      else xp16_tiles[blk - (nblocks - 1)])
                # Interior: xp rows/cols 1..8 <- x, per phase and parity.
                # x row 2i+hl-1, col 2k+wl-1; gather-read from x_sb (batch
                # is outermost there), contiguous write into xp_ph.
                xv = x_sb.rearrange("p b j (h w) -> p j h w b", w=8)
                # Emit in first-use order (phase (1,1) feeds the very first
                # matmuls of every group); only the 3 used phases are built.
                ei = 0
                for hl, wl in USED_PHASES:
                    i0 = 1 - hl
                    k0 = 1 - wl
                    for j in range(2):
                        src = xv[:, j, (1 - hl)::2, (1 - wl)::2, :]
                        dst = xp[:, j, hl, wl, i0:i0 + 4, k0:k0 + 4, :]
                        # Block 0's assembly is on the critical path to the
                        # first matmul: split it across Vector and the
                        # (still idle) Scalar engine.
                        if blk == 0 and ei in (1, 3):
                            nc.scalar.copy(out=dst, in_=src)
                        else:
                            nc.vector.tensor_copy(out=dst, in_=src)
                        ei += 1
                # Boundary fixups: strided runs on the flat 5x5 phase grids.
                xpf = xp.rearrange("p j hl wl i k b -> p j hl wl (i k) b")
                xsf = x_sb.rearrange("p b j f -> p j f b")
                for (hl, wl, d0, dd, s0, ds, cnt) in PHASE_FIXUPS:
                    nc.vector.tensor_copy(
                        out=xpf[:, :, hl, wl, d0 : d0 + dd * (cnt - 1) + 1 : dd, :],
                        in_=xsf[:, :, s0 : s0 + ds * (cnt - 1) + 1 : ds, :],
                    )

                osb = osb_tiles[blk % 2]
                ob = osb.rearrange("p b g (h w) -> p b g h w", w=8)
                for ci, taps in enumerate(CONV_TAPS):
                    for g in range(2):
                        ps = pspool.tile([128, bs * 16], F32, name="ps", tag="ps")
                        nmm = len(taps) * 2
                        k = 0
                        for (dr, dc) in taps:
                            pair_idx = PAIRS.index((ci, dr, dc))
                            for j in range(2):
                                # window rows dr//2..+3 of parity dr%2, cols
                                # dc//2..+3 of parity dc%2; free order (i,k,b)
                                rhs = xp[
                                    :, j, dr % 2, dc % 2,
                                    dr // 2 : dr // 2 + 4,
                                    dc // 2 : dc // 2 + 4,
                                    :,
                                ]
                                nc.tensor.matmul(
                                    ps[:],
                                    lhsT=w_sb[:, pair_idx, j, g],
                                    rhs=rhs,
                                    start=(k == 0),
                                    stop=(k == nmm - 1),
                                )
                                k += 1
                        # Drain with fused bias into interleaved position,
                        # split per batch-half so the output DMA of the first
                        # half overlaps the second half's drains (shrinks the
                        # end-of-kernel tail). PSUM free order is (i, k, b).
                        ps3 = ps.rearrange("p (i k b) -> p b i k", i=4, k=4)
                        if ci == 0:  # up -> (2i, 2j)
                            dst = ob[:, :, g, 0:8:2, 0:8:2]
                        elif ci == 1:  # left -> (2i+1, 2j)
                            dst = ob[:, :, g, 1:8:2, 0:8:2]
                        else:  # right -> (2i+1, 2j+1)
                            dst = ob[:, :, g, 1:8:2, 1:8:2]
                        bh = bs // 2
                        for h in range(2):
                            nc.scalar.add(
                                out=dst[:, h * bh : (h + 1) * bh],
                                in_=ps3[:, h * bh : (h + 1) * bh],
                                add=bias_sb[:, ci * 2 + g : ci * 2 + g + 1],
                            )

                # Scatter-zeros + store, per batch-half (512B DMA runs).
                # Outputs ride the scalar ring; the last blocks' second
                # halves move to the sync ring, which is idle once the input
                # stream has drained — this pulls in the end-of-kernel tail.
                bh = bs // 2
                for h in range(2):
                    for (a, b_, s) in OUT_ZERO_RUNS:
                        nc.gpsimd.memset(
                            osb[:, h * bh : (h + 1) * bh, :, a:b_:s], 0.0
                        )
                    ring = nc.scalar
                    if h == 1 and blk >= len(plan) - 3:
                        ring = nc.sync
                    ring.dma_start(
                        out=oap[b0 + h * bh : b0 + (h + 1) * bh].rearrange(
                            "b (p g) h w -> p b g (h w)", g=2
                        ),
                        in_=osb[:, h * bh : (h + 1) * bh],
                    )

    nc.compile()
    return nc


def prep_weights(w_up, b_up, w_left, b_left, w_right, b_right):
    ws = [np.asarray(w_up), np.asarray(w_left), np.asarray(w_right)]
    bs = [np.asarray(b_up), np.asarray(b_left), np.asarray(b_right)]
    wt = np.empty((128, NPAIRS, 2, 2, 128), np.float32)
    for t, (ci, dr, dc) in enumerate(PAIRS):
        w = ws[ci][:, :, dr, dc]  # [O, I]
        # wt[p, t, j, g, m] = w[2m+g, 2p+j]
        w4 = w.reshape(128, 2, 128, 2)  # [m, g, p, j]
        wt[:, t] = w4.transpose(2, 3, 1, 0)
    # Padded to 512B per partition so the bias DMA uses full-size descriptors.
    bias = np.zeros((128, 128), np.float32)
    for ci, b in enumerate(bs):
        b2 = b.reshape(128, 2)  # [p, g] -> channel 2p+g
        bias[:, ci * 2 + 0] = b2[:, 0]
        bias[:, ci * 2 + 1] = b2[:, 1]
    import ml_dtypes
    wt = wt.astype(ml_dtypes.bfloat16)
    return np.ascontiguousarray(wt), np.ascontiguousarray(bias)


_NC_CACHE = {}


def _get_nc(**kw):
    key = tuple(sorted(kw.items()))
    if key not in _NC_CACHE:
        _NC_CACHE[key] = build_nc(**kw)
    return _NC_CACHE[key]


def run(inputs, trace=False, **build_kw):
    x = np.asarray(inputs["x"], dtype=np.float32)
    assert x.shape == (B_FULL, CIN, 8, 8), x.shape
    wt, bias = prep_weights(
        inputs["w_up"], inputs["b_up"], inputs["w_left"], inputs["b_left"],
        inputs["w_right"], inputs["b_right"],
    )
    nc = _get_nc(**build_kw)
    in_maps = [
        {
            "x": np.ascontiguousarray(x[i * B_CORE : (i + 1) * B_CORE]),
            "wt": wt,
            "bias": bias,
        }
        for i in range(N_CORES)
    ]
    res = run_bass_kernel_spmd(nc, in_maps, core_ids=list(range(N_CORES)), trace=trace)
    out = np.concatenate([r["out"] for r in res.results], axis=0)
    return out, res


def kernel(**inputs):
    out, _ = run(inputs)
    return out
